# revision 1
# baseline (speedup 1.0000x reference)
"""Trainium2 Bass kernel for a 3-layer binarized MLP (MNIST BNN, eval mode).

Math (per layer): z = ((h @ sign(W).T + b) - m) * g/sqrt(v+eps) + be
layers 1,2 then binarize (sign); layer 3 returns logits.

Device strategy (data-parallel over 8 cores, 4096 batch rows each):
  - activations kept feature-major (transposed) so the contraction dim is
    always on SBUF partitions; final [10, B] output transposed on host.
  - L1: x.T split into two fp16 planes (hi + residual) -> 2 matmuls vs
    binarized W1 (+-1 in fp16). fp32-class accuracy at 2x bf16-matmul cost.
  - binarize+BN folded into a per-channel threshold: t = (psum >= thr),
    computed by one DVE tensor_scalar(is_ge) per PSUM tile, output {0,1} fp8.
  - L2/L3 weights are 2*sign(W) in fp8 (exact); the sign-rowsum correction is
    folded into the next layer's threshold / output bias. All integer values
    accumulate exactly in fp32 PSUM.
  - L3: logits = psum * alpha + beta via ScalarE activation (per-partition
    scale/bias), DMA'd out as [10, 4096] fp32.
"""

import numpy as np
import ml_dtypes
from contextlib import ExitStack

import concourse.bass as bass
import concourse.tile as tile
import concourse.mybir as mybir
from concourse import bacc

P = 128
B = 32768
B_CORE = 4096
D_IN = 784
D_FULL = 768           # 6 full 128-row k-tiles for layer 1
K1 = D_FULL // P       # 6
D_TAIL = D_IN - D_FULL  # 16 leftover rows; hi+lo tails merged into one K=32 mm
H = 1024
KH = H // P            # 8 k-tiles / h-tiles for hidden layers
D_OUT = 10
M_PAD = 16             # padded output-feature count
N_CORES = 8
NMM = 512              # matmul moving free dim / PSUM bank (fp32)
BCH = 1024             # layer-1 batch chunk (DMA double buffering)
L2_S_INNER = True      # layer-2 loop order: s innermost (stationary reuse)
XPOOL_BUFS = 2         # layer-1 input chunk double/triple buffering
L2_ACT_SIGN = False    # layer-2 evac on ScalarE (Sign -> +-1) instead of DVE

F32 = mybir.dt.float32
F16 = mybir.dt.float16
F8 = mybir.dt.float8e4

NP_F8 = mybir.dt.np(F8)   # ml_dtypes.float8_e4m3

TRACE = False
LAST_RESULTS = None
USE_DR = True   # fp8 DoubleRow for layers 2/3

_cached = None
_runner = None


def _build_nc(repeat=1, hw_loop=False):
    """Build + compile the single-core SPMD program. Returns the Bacc.

    repeat>1 replicates the whole compute body (used only for benchmarking:
    on-device time per iteration = slope of exec time vs repeat).
    hw_loop=True uses a hardware For_i loop instead of unrolling.
    """
    nc = bacc.Bacc(
        "TRN2",
        target_bir_lowering=False,
        debug=False,
        enable_asserts=False,
        num_devices=1,
    )

    xh_d = nc.dram_tensor("xh", [D_FULL, B_CORE], F16, kind="ExternalInput").ap()
    xl_d = nc.dram_tensor("xl", [D_FULL, B_CORE], F16, kind="ExternalInput").ap()
    xt_d = nc.dram_tensor("xt", [2 * D_TAIL, B_CORE], F16, kind="ExternalInput").ap()
    w1_d = nc.dram_tensor("w1t", [D_FULL, H], F16, kind="ExternalInput").ap()
    w1tail_d = nc.dram_tensor("w1tail", [2 * D_TAIL, H], F16, kind="ExternalInput").ap()
    w2_d = nc.dram_tensor("w2t", [H, H], F8, kind="ExternalInput").ap()
    w3_d = nc.dram_tensor("w3t", [H, M_PAD], F8, kind="ExternalInput").ap()
    thr1_d = nc.dram_tensor("thr1", [H], F32, kind="ExternalInput").ap()
    thr2_d = nc.dram_tensor("thr2", [H], F32, kind="ExternalInput").ap()
    a2_d = nc.dram_tensor("a2", [H], F32, kind="ExternalInput").ap()
    b2v_d = nc.dram_tensor("b2v", [H], F32, kind="ExternalInput").ap()
    a3_d = nc.dram_tensor("a3", [M_PAD], F32, kind="ExternalInput").ap()
    b3_d = nc.dram_tensor("b3", [M_PAD], F32, kind="ExternalInput").ap()
    out_d = nc.dram_tensor("out", [D_OUT, B_CORE], F32, kind="ExternalOutput").ap()

    xh_r = xh_d.rearrange("(ko p) b -> p ko b", p=P)
    xl_r = xl_d.rearrange("(ko p) b -> p ko b", p=P)

    with tile.TileContext(nc) as tc, ExitStack() as ctx:
        consts = ctx.enter_context(tc.tile_pool(name="consts", bufs=1))
        xpool = ctx.enter_context(tc.tile_pool(name="xin", bufs=XPOOL_BUFS))
        hbuf = ctx.enter_context(tc.tile_pool(name="hbuf", bufs=1))
        psum = ctx.enter_context(tc.tile_pool(name="ps", bufs=8, space="PSUM"))
        opool = ctx.enter_context(tc.tile_pool(name="opool", bufs=2))

        w1 = consts.tile([P, K1, H], F16)
        nc.sync.dma_start(w1[:], w1_d.rearrange("(ko p) h -> p ko h", p=P))
        wtail = consts.tile([2 * D_TAIL, H], F16)
        nc.sync.dma_start(wtail[:], w1tail_d)
        w2 = consts.tile([P, KH, H], F8)
        nc.sync.dma_start(w2[:], w2_d.rearrange("(ko p) h -> p ko h", p=P))
        w3 = consts.tile([P, KH, M_PAD], F8)
        nc.sync.dma_start(w3[:], w3_d.rearrange("(ko p) h -> p ko h", p=P))
        thr1 = consts.tile([P, KH], F32)
        nc.sync.dma_start(thr1[:], thr1_d.rearrange("(ko p) -> p ko", p=P))
        thr2 = consts.tile([P, KH], F32)
        nc.sync.dma_start(thr2[:], thr2_d.rearrange("(ko p) -> p ko", p=P))
        a2t = consts.tile([P, KH], F32)
        nc.sync.dma_start(a2t[:], a2_d.rearrange("(ko p) -> p ko", p=P))
        b2vt = consts.tile([P, KH], F32)
        nc.sync.dma_start(b2vt[:], b2v_d.rearrange("(ko p) -> p ko", p=P))
        a3 = consts.tile([M_PAD, 1], F32)
        nc.sync.dma_start(a3[:], a3_d.rearrange("(m o) -> m o", o=1))
        b3 = consts.tile([M_PAD, 1], F32)
        nc.sync.dma_start(b3[:], b3_d.rearrange("(m o) -> m o", o=1))

        def emit_body():
            t1 = hbuf.tile([P, KH, B_CORE], F8, tag="t1")
            t2 = hbuf.tile([P, KH, B_CORE], F8, tag="t2")

            # ---- Layer 1: mm1 = x @ sign(W1).T ; t1 = mm1 >= thr1 ----
            for c in range(B_CORE // BCH):
                b0 = c * BCH
                xh = xpool.tile([P, K1, BCH], F16, tag="xh")
                nc.sync.dma_start(xh[:], xh_r[:, :, b0 : b0 + BCH])
                xl = xpool.tile([P, K1, BCH], F16, tag="xl")
                nc.sync.dma_start(xl[:], xl_r[:, :, b0 : b0 + BCH])
                xt = xpool.tile([2 * D_TAIL, BCH], F16, tag="xt")
                nc.sync.dma_start(xt[:], xt_d[:, b0 : b0 + BCH])
                for h in range(KH):
                    for s in range(BCH // NMM):
                        n0 = s * NMM
                        ps = psum.tile([P, NMM], F32, tag="ps")
                        for k in range(K1):
                            nc.tensor.matmul(
                                ps[:],
                                w1[:, k, h * P : (h + 1) * P],
                                xh[:, k, n0 : n0 + NMM],
                                start=(k == 0),
                                stop=False,
                            )
                            nc.tensor.matmul(
                                ps[:],
                                w1[:, k, h * P : (h + 1) * P],
                                xl[:, k, n0 : n0 + NMM],
                                start=False,
                                stop=False,
                            )
                        # merged hi+lo tail: one K=32 matmul
                        nc.tensor.matmul(
                            ps[:],
                            wtail[:, h * P : (h + 1) * P],
                            xt[:, n0 : n0 + NMM],
                            start=False,
                            stop=True,
                        )
                        nc.vector.tensor_scalar(
                            out=t1[:, h, b0 + n0 : b0 + n0 + NMM],
                            in0=ps[:],
                            scalar1=thr1[:, h : h + 1],
                            scalar2=None,
                            op0=mybir.AluOpType.is_ge,
                        )

            # ---- Layer 2: mm2 = (2*sign(W2)) @ t1 ; t2 = mm2 >= thr2' ----
            if L2_S_INNER:
                # s innermost: each DR stationary feeds 8 consecutive MMs,
                # 8 PSUM banks accumulate in parallel
                assert USE_DR
                for h in range(KH):
                    pss = [psum.tile([P, NMM], F32, tag="ps", name=f"ps2_{h}_{i}")
                           for i in range(B_CORE // NMM)]
                    for k in range(0, KH, 2):
                        for s in range(B_CORE // NMM):
                            nc.tensor.matmul(
                                pss[s][:],
                                w2[:, k : k + 2, h * P : (h + 1) * P],
                                t1[:, k : k + 2, s * NMM : (s + 1) * NMM],
                                perf_mode=mybir.MatmulPerfMode.DoubleRow,
                                start=(k == 0),
                                stop=(k == KH - 2),
                            )
                    for s in range(B_CORE // NMM):
                        if L2_ACT_SIGN:
                            nc.scalar.activation(
                                t2[:, h, s * NMM : (s + 1) * NMM],
                                pss[s][:],
                                mybir.ActivationFunctionType.Sign,
                                bias=b2vt[:, h : h + 1],
                                scale=a2t[:, h : h + 1],
                            )
                        else:
                            nc.vector.tensor_scalar(
                                out=t2[:, h, s * NMM : (s + 1) * NMM],
                                in0=pss[s][:],
                                scalar1=thr2[:, h : h + 1],
                                scalar2=None,
                                op0=mybir.AluOpType.is_ge,
                            )
            for h in range(KH if not L2_S_INNER else 0):
                for s in range(B_CORE // NMM):
                    n0 = s * NMM
                    ps = psum.tile([P, NMM], F32, tag="ps")
                    if USE_DR:
                        for k in range(0, KH, 2):
                            nc.tensor.matmul(
                                ps[:],
                                w2[:, k : k + 2, h * P : (h + 1) * P],
                                t1[:, k : k + 2, n0 : n0 + NMM],
                                perf_mode=mybir.MatmulPerfMode.DoubleRow,
                                start=(k == 0),
                                stop=(k == KH - 2),
                            )
                    else:
                        for k in range(KH):
                            nc.tensor.matmul(
                                ps[:],
                                w2[:, k, h * P : (h + 1) * P],
                                t1[:, k, n0 : n0 + NMM],
                                start=(k == 0),
                                stop=(k == KH - 1),
                            )
                    nc.vector.tensor_scalar(
                        out=t2[:, h, n0 : n0 + NMM],
                        in0=ps[:],
                        scalar1=thr2[:, h : h + 1],
                        scalar2=None,
                        op0=mybir.AluOpType.is_ge,
                    )

            # ---- Layer 3: logits = (2*sign(W3)) @ t2 * alpha3 + beta3 ----
            for s in range(B_CORE // NMM):
                n0 = s * NMM
                ps = psum.tile([P, NMM], F32, tag="ps")
                if USE_DR:
                    for k in range(0, KH, 2):
                        nc.tensor.matmul(
                            ps[:D_OUT],
                            w3[:, k : k + 2, :D_OUT],
                            t2[:, k : k + 2, n0 : n0 + NMM],
                            perf_mode=mybir.MatmulPerfMode.DoubleRow,
                            start=(k == 0),
                            stop=(k == KH - 2),
                        )
                else:
                    for k in range(KH):
                        nc.tensor.matmul(
                            ps[:D_OUT],
                            w3[:, k, :D_OUT],
                            t2[:, k, n0 : n0 + NMM],
                            start=(k == 0),
                            stop=(k == KH - 1),
                        )
                ot = opool.tile([M_PAD, NMM], F32, tag="ot")
                nc.scalar.activation(
                    ot[:D_OUT],
                    ps[:D_OUT],
                    mybir.ActivationFunctionType.Identity,
                    bias=b3[:D_OUT],
                    scale=a3[:D_OUT],
                )
                nc.sync.dma_start(out_d[:, n0 : n0 + NMM], ot[:D_OUT])

        if hw_loop and repeat > 1:
            with tc.For_i(0, repeat, 1):
                emit_body()
        else:
            for _rep in range(repeat):
                emit_body()

    nc.compile()
    return nc


def _prep_inputs(x, W1, b1, g1, be1, m1, v1, W2, b2, g2, be2, m2, v2,
                 W3, b3, g3, be3, m3, v3):
    """Host-side preprocessing: fold BN into thresholds, binarize weights,
    split x.T into two fp16 planes. Returns (shared_map, per_core_xplanes)."""
    x, W1, W2, W3 = (np.asarray(a, np.float32) for a in (x, W1, W2, W3))
    b1, g1, be1, m1, v1 = (np.asarray(a, np.float32) for a in (b1, g1, be1, m1, v1))
    b2, g2, be2, m2, v2 = (np.asarray(a, np.float32) for a in (b2, g2, be2, m2, v2))
    b3, g3, be3, m3, v3 = (np.asarray(a, np.float32) for a in (b3, g3, be3, m3, v3))
    eps = 1e-5

    def inv_of(g, v):
        return g.astype(np.float64) / np.sqrt(v.astype(np.float64) + eps)

    def thr_of(b, g, be, m, v, extra=0.0):
        # z >= 0  <=>  mm >= (m - b) - be/inv  (+ extra rowsum correction)
        inv = inv_of(g, v)
        num = be.astype(np.float64)
        safe = inv > 0
        t = np.where(
            safe,
            (m.astype(np.float64) - b.astype(np.float64))
            - num / np.where(safe, inv, 1.0),
            np.where(num >= 0, -1e30, 1e30),
        )
        return (t + extra).astype(np.float32)

    s1 = np.where(W1 >= 0, np.float32(1.0), np.float32(-1.0))  # [H, D_IN]
    s2 = np.where(W2 >= 0, np.float32(1.0), np.float32(-1.0))  # [H, H]
    s3 = np.where(W3 >= 0, np.float32(1.0), np.float32(-1.0))  # [D_OUT, H]

    w1t_full = s1.T.astype(np.float16)                         # [D_IN, H]
    w1t = np.ascontiguousarray(w1t_full[:D_FULL])
    w1tail = np.concatenate([w1t_full[D_FULL:], w1t_full[D_FULL:]], axis=0)

    w2t = np.ascontiguousarray((2.0 * s2.T)).astype(NP_F8)     # [H, H]
    # h2 coding: {0,1} (DVE is_ge) needs W3 doubled + rowsum in beta3;
    # +-1 (ACT Sign) uses plain sign(W3) and no rowsum.
    w3_scale = 1.0 if L2_ACT_SIGN else 2.0
    w3t = np.zeros((H, M_PAD), NP_F8)
    w3t[:, :D_OUT] = (w3_scale * s3.T).astype(NP_F8)

    thr1 = thr_of(b1, g1, be1, m1, v1)
    r2 = s2.sum(axis=1, dtype=np.float64)                      # [H]
    thr2 = thr_of(b2, g2, be2, m2, v2, extra=r2)
    inv2 = inv_of(g2, v2)
    a2 = inv2.astype(np.float32)
    b2v = (
        (b2.astype(np.float64) - m2.astype(np.float64) - r2) * inv2
        + be2.astype(np.float64)
    ).astype(np.float32)

    inv3 = inv_of(g3, v3)
    r3 = s3.sum(axis=1, dtype=np.float64)                      # [D_OUT]
    alpha3 = np.zeros(M_PAD, np.float32)
    alpha3[:D_OUT] = inv3.astype(np.float32)
    beta3 = np.zeros(M_PAD, np.float32)
    r3_term = 0.0 if L2_ACT_SIGN else r3
    beta3[:D_OUT] = (
        (b3.astype(np.float64) - m3.astype(np.float64) - r3_term) * inv3
        + be3.astype(np.float64)
    ).astype(np.float32)

    # x.T split into fp16 hi + residual planes; last 16 rows of each plane
    # merged into one 32-row tail plane (hi rows stacked on lo rows).
    xT = np.ascontiguousarray(x.T.astype(np.float32))          # [D_IN, B]
    xh_full = xT.astype(np.float16)
    xl_full = (xT - xh_full.astype(np.float32)).astype(np.float16)
    xtail = np.concatenate([xh_full[D_FULL:], xl_full[D_FULL:]], axis=0)

    shared = {
        "w1t": w1t,
        "w1tail": w1tail,
        "w2t": w2t,
        "w3t": w3t,
        "thr1": thr1,
        "thr2": thr2,
        "a2": a2,
        "b2v": b2v,
        "a3": alpha3,
        "b3": beta3,
    }
    planes = []
    for i in range(N_CORES):
        sl = slice(i * B_CORE, (i + 1) * B_CORE)
        planes.append(
            {
                "xh": np.ascontiguousarray(xh_full[:D_FULL, sl]),
                "xl": np.ascontiguousarray(xl_full[:D_FULL, sl]),
                "xt": np.ascontiguousarray(xtail[:, sl]),
            }
        )
    return shared, planes


class _Runner:
    """Persistent PJRT runner for the compiled Bass program on 8 cores.

    Mirrors concourse.bass2jax.run_bass_via_pjrt's multi-core path, but keeps
    the jitted shard_map callable and committed device inputs alive so
    repeated executions neither re-trace nor re-transfer inputs.
    """

    def __init__(self, nc):
        import jax
        from jax.experimental.shard_map import shard_map
        from jax.sharding import Mesh, PartitionSpec, NamedSharding
        from concourse.bass2jax import (
            install_neuronx_cc_hook,
            _bass_exec_p,
            partition_id_tensor,
        )

        install_neuronx_cc_hook()
        self.jax = jax
        self.nc = nc
        partition_name = (
            nc.partition_id_tensor.name if nc.partition_id_tensor else None
        )
        in_names, out_names, out_avals = [], [], []
        for alloc in nc.m.functions[0].allocations:
            if not isinstance(alloc, mybir.MemoryLocationSet):
                continue
            name = alloc.memorylocations[0].name
            if alloc.kind == "ExternalInput":
                if name != partition_name:
                    in_names.append(name)
            elif alloc.kind == "ExternalOutput":
                out_names.append(name)
                out_avals.append(
                    jax.core.ShapedArray(
                        tuple(alloc.tensor_shape), mybir.dt.np(alloc.dtype)
                    )
                )
        self.in_names = in_names
        self.out_names = out_names
        self.out_avals = out_avals
        n_params = len(in_names)
        n_outs = len(out_names)
        bind_names = in_names + out_names
        if partition_name is not None:
            bind_names = bind_names + [partition_name]
        bind_names = tuple(bind_names)

        def _body(*args):
            operands = list(args)
            if partition_name is not None:
                operands.append(partition_id_tensor())
            outs = _bass_exec_p.bind(
                *operands,
                out_avals=tuple(out_avals),
                in_names=bind_names,
                out_names=tuple(out_names),
                lowering_input_output_aliases=(),
                sim_require_finite=True,
                sim_require_nnan=True,
                nc=nc,
            )
            return tuple(outs)

        devices = jax.devices()[:N_CORES]
        assert len(devices) == N_CORES, devices
        self.mesh = Mesh(np.asarray(devices), ("core",))
        self.sharding = NamedSharding(self.mesh, PartitionSpec("core"))
        self.sharded = jax.jit(
            shard_map(
                _body,
                mesh=self.mesh,
                in_specs=(PartitionSpec("core"),) * (n_params + n_outs),
                out_specs=(PartitionSpec("core"),) * n_outs,
                check_rep=False,
            ),
            donate_argnums=tuple(range(n_params, n_params + n_outs)),
            keep_unused=True,
        )

    def put_inputs(self, in_maps):
        """Concat per-core inputs on axis 0 and commit them to the mesh."""
        concat = [
            np.concatenate([np.asarray(m[name]) for m in in_maps], axis=0)
            for name in self.in_names
        ]
        return [self.jax.device_put(a, self.sharding) for a in concat]

    def zero_outs(self):
        return [
            self.jax.device_put(
                np.zeros((N_CORES * a.shape[0], *a.shape[1:]), a.dtype),
                self.sharding,
            )
            for a in self.out_avals
        ]

    def execute(self, dev_in):
        outs = self.sharded(*dev_in, *self.zero_outs())
        self.jax.block_until_ready(outs)
        return outs

    def outputs_to_np(self, outs):
        return [
            {
                name: np.asarray(outs[i]).reshape(
                    N_CORES, *self.out_avals[i].shape
                )[c]
                for i, name in enumerate(self.out_names)
            }
            for c in range(N_CORES)
        ]


def _get_runner():
    global _cached, _runner
    if _runner is None:
        if _cached is None:
            _cached = _build_nc()
        _runner = _Runner(_cached)
    return _runner


def kernel(**inputs):
    runner = _get_runner()
    shared, planes = _prep_inputs(**inputs)
    in_maps = [{**shared, **planes[i]} for i in range(N_CORES)]
    dev_in = runner.put_inputs(in_maps)
    outs = runner.execute(dev_in)
    results = runner.outputs_to_np(outs)

    out = np.empty((B, D_OUT), np.float32)
    for i in range(N_CORES):
        out[i * B_CORE : (i + 1) * B_CORE] = results[i]["out"].T
    return out

